# revision 8
# baseline (speedup 1.0000x reference)
"""Cross multi-head attention Trainium2 Bass kernel (v2).

Problem: nn_CrossMutiHeadAttention (B=4, SQ=SKV=2048, d_model=1024, H=8,
d_k=64, d_v=128), fp32 in/out.

Sharding (8 cores, no collectives): core c handles batch c//2 and query-row
half c%2 — each core computes K/V projections for its batch (duplicated
across the 2 cores sharing a batch) plus attention + output projection for
its 1024 query rows.

v2 changes vs v1 (422us baseline):
  - DMA: consolidated into few large transfers; SWDGE (gpsimd) cast-DMAs
    convert f32->bf16 in flight, eliminating staging copies on DVE.
  - scores: head-pair packed K^T/Q^T lets the two K=64 score matmuls of a
    pair run on disjoint PE row-groups (tile_position (0,0)/(64,0)) -
    issued adjacently they execute concurrently (~2x scores throughput).
  - softmax denominator: M=1 ones-matmuls accumulate into psum partitions
    0 (head a) / 64 (head b) via disjoint col-groups - adjacent issue makes
    the pair concurrent; replaces the full-rate [1,512] ones-matmul per
    chunk (8192 -> ~4130 cycles per unit).
  - reciprocal: RECIPROCAL_APPROX_FAST custom DVE op (~1cpe vs 6cpe).
  - Wo and O^T held in bf16 (saves 32KB/partition SBUF + enables FWL).
  - ph5 emitted per query-half so its matmuls can fill PE gaps while ph4's
    ACT exp stream (the ph4 bottleneck) catches up.
"""

from contextlib import ExitStack

import numpy as np

import concourse.bass as bass
import concourse.mybir as mybir
from concourse import bacc
from concourse.bass_utils import run_bass_kernel_spmd
from concourse.masks import make_identity
from concourse.tile import TileContext

F32 = mybir.dt.float32
BF16 = mybir.dt.bfloat16

P = 128
B, SQ, SKV, DM = 4, 2048, 2048, 1024
H, DK, DV = 8, 64, 128
SQH = SQ // 2          # 1024 query rows per core
HP = H // 2            # 4 head pairs
CO = DM // P           # 8 contraction chunks
N_CORES = 8

EXP_SCALE = 1.0 / np.sqrt(DK).astype(np.float32)  # 0.125


def build(loop_phase="all"):
    nc = bacc.Bacc()
    enc = nc.declare_dram_parameter("enc", [SKV, DM], F32, isOutput=False)
    pre = nc.declare_dram_parameter("pre", [SQH, DM], F32, isOutput=False)
    wq = nc.declare_dram_parameter("wq", [DM, H * DK], F32, isOutput=False)
    wk = nc.declare_dram_parameter("wk", [DM, H * DK], F32, isOutput=False)
    wv = nc.declare_dram_parameter("wv", [DM, DM], F32, isOutput=False)
    wo = nc.declare_dram_parameter("wo", [DM, DM], F32, isOutput=False)
    n_it = nc.declare_dram_parameter("n_it", [1, 1], mybir.dt.uint32, isOutput=False)
    out = nc.declare_dram_parameter("out", [SQH, DM], F32, isOutput=True)

    with ExitStack() as ctx:
        tc = ctx.enter_context(TileContext(nc))
        ec = ctx.enter_context
        if True:
            cpool = ec(tc.tile_pool(name="const", bufs=1))
            stage_pool = ec(tc.tile_pool(name="stage", bufs=2))
            trn_pool = ec(tc.tile_pool(name="trn", bufs=2))
            wk_pool = ec(tc.tile_pool(name="wk", bufs=1))
            wq_pool = ec(tc.tile_pool(name="wq", bufs=1))
            wv_pool = ec(tc.tile_pool(name="wv", bufs=1))
            wo_pool = ec(tc.tile_pool(name="wo", bufs=1))
            kt_pool = ec(tc.tile_pool(name="kt", bufs=1))
            qt_pool = ec(tc.tile_pool(name="qt", bufs=1))
            v_pool = ec(tc.tile_pool(name="vpool", bufs=1))
            exp_pool = ec(tc.tile_pool(name="exp", bufs=6))
            ot_pool = ec(tc.tile_pool(name="ot", bufs=1))
            r_pool = ec(tc.tile_pool(name="rsm", bufs=2))
            rb_pool = ec(tc.tile_pool(name="rb", bufs=4))
            y_pool = ec(tc.tile_pool(name="ysb", bufs=2))
            # PSUM: ps_st 2x[P,2,512]f32 = 4 banks (scores; also ph1/ph5 via
            # smaller tags), ps_ot 1x[P,2,512] = 2 banks, ps_dn 1 bank -> 7/8
            ps_st = ec(tc.tile_pool(name="ps_st", bufs=2, space="PSUM"))
            ps_ot = ec(tc.tile_pool(name="ps_ot", bufs=1, space="PSUM"))
            ps_dn = ec(tc.tile_pool(name="ps_dn", bufs=1, space="PSUM"))

            ident = cpool.tile([P, P], BF16, tag="ident")
            make_identity(nc, ident[:])
            ones = cpool.tile([P, 1], BF16, tag="ones")
            nc.gpsimd.memset(ones[:], 1.0)
            F32R = mybir.dt.float32r
            ones_row_f = cpool.tile([P, P], F32, tag="ones_row_f")
            nc.gpsimd.memset(ones_row_f[:], 1.0)
            ones_row = cpool.tile([P, P], F32R, tag="ones_row")
            nc.vector.tensor_copy(ones_row[:], ones_row_f[:])
            nit_sb = cpool.tile([1, 1], mybir.dt.uint32, tag="nit")
            nc.sync.dma_start(nit_sb[:], n_it[:])

            regs = []
            for eng_t in mybir.ALL_ENGINES:
                r = nc.alloc_register(eng_t, f"nit_{eng_t.name}")
                nc.engines[eng_t].reg_load(r, nit_sb[0:1, 0:1])
                regs.append(r)
            n_val = bass.RegisterHandles(regs)

            state = {}

            def load_weights_kv():
                # one cast-DMA per weight matrix (SWDGE converts f32->bf16
                # in flight; no staging, no DVE cast ops)
                wk_sb = wk_pool.tile([P, CO, H * DK], BF16, tag="wk")
                nc.gpsimd.dma_start(
                    wk_sb[:], wk.rearrange("(co p) n -> p co n", p=P)
                )
                wv_sb = wv_pool.tile([P, CO, DM], BF16, tag="wv")
                nc.gpsimd.dma_start(
                    wv_sb[:], wv.rearrange("(co p) n -> p co n", p=P)
                )
                state["wk_sb"], state["wv_sb"] = wk_sb, wv_sb

            def load_weights_qo():
                wq_sb = wq_pool.tile([P, CO, H * DK], BF16, tag="wq")
                nc.gpsimd.dma_start(
                    wq_sb[:], wq.rearrange("(co p) n -> p co n", p=P)
                )
                wo_sb = wo_pool.tile([P, CO, DM], BF16, tag="wo")
                nc.gpsimd.dma_start(
                    wo_sb[:], wo.rearrange("(co p) n -> p co n", p=P)
                )
                state["wq_sb"], state["wo_sb"] = wq_sb, wo_sb

            def transpose_block(src, et):
                # src [P, 4, DM] bf16 (rows r of 512-block at [r%128, r//128])
                # -> et [P, CO, 512] bf16 (dm on partitions, 512 rows free)
                for co in range(CO):
                    tp = ps_st.tile([P, 4, P], BF16, tag="st")
                    for t in range(4):
                        nc.tensor.transpose(
                            tp[:, t, :],
                            src[:, t, co * P : (co + 1) * P],
                            ident[:],
                        )
                    nc.vector.tensor_copy(et[:, co, :], tp[:])

            def ph1():
                kt_sb = kt_pool.tile([P, HP, SKV], BF16, tag="kt")
                v_sb = v_pool.tile([P, SKV // P, DM], BF16, tag="v")
                state["kt_sb"], state["v_sb"] = kt_sb, v_sb
                stgs = {}
                for blk in range(SKV // 512):
                    # blk0's DMA ahead of the weight loads (single SWDGE FIFO)
                    stg = stage_pool.tile([P, 4, DM], BF16, tag="stage")
                    nc.gpsimd.dma_start(
                        stg[:],
                        enc.rearrange("(b t p) n -> p b t n", p=P, t=4)[
                            :, blk, :, :
                        ],
                    )
                    stgs[blk] = stg
                    if blk == 0:
                        load_weights_kv()
                wk_sb, wv_sb = state["wk_sb"], state["wv_sb"]
                for blk in range(SKV // 512):
                    stg = stgs[blk]
                    et = trn_pool.tile([P, CO, 512], BF16, tag="trn")
                    transpose_block(stg, et)
                    # K^T proj: [128 pair-d, 512 kv] per head pair
                    for hp in range(HP):
                        kp = ps_st.tile([P, 512], F32, tag="st")
                        for co in range(CO):
                            nc.tensor.matmul(
                                kp[:],
                                lhsT=wk_sb[:, co, hp * P : (hp + 1) * P],
                                rhs=et[:, co, :],
                                start=(co == 0),
                                stop=(co == CO - 1),
                            )
                        nc.scalar.copy(
                            kt_sb[:, hp, blk * 512 : (blk + 1) * 512], kp[:]
                        )
                    # V proj: [128 kv, 512 dv]
                    for dvh in range(2):
                        for t in range(4):
                            vp = ps_st.tile([P, 512], F32, tag="st")
                            for co in range(CO):
                                nc.tensor.matmul(
                                    vp[:],
                                    lhsT=et[:, co, t * P : (t + 1) * P],
                                    rhs=wv_sb[:, co, dvh * 512 : (dvh + 1) * 512],
                                    start=(co == 0),
                                    stop=(co == CO - 1),
                                )
                            dst = v_sb[:, blk * 4 + t, dvh * 512 : (dvh + 1) * 512]
                            if t % 2 == 0:
                                nc.vector.tensor_copy(dst, vp[:])
                            else:
                                nc.scalar.copy(dst, vp[:])

            def ph23():
                load_weights_qo()
                wq_sb = state["wq_sb"]
                qt_sb = qt_pool.tile([P, HP, SQH], BF16, tag="qt")
                state["qt_sb"] = qt_sb
                for qc in range(2):
                    stg = stage_pool.tile([P, 4, DM], BF16, tag="stage")
                    nc.gpsimd.dma_start(
                        stg[:],
                        pre.rearrange("(b t p) n -> p b t n", p=P, t=4)[
                            :, qc, :, :
                        ],
                    )
                    pt = trn_pool.tile([P, CO, 512], BF16, tag="trn")
                    transpose_block(stg, pt)
                    for hp in range(HP):
                        qp = ps_st.tile([P, 512], F32, tag="st")
                        for co in range(CO):
                            nc.tensor.matmul(
                                qp[:],
                                lhsT=wq_sb[:, co, hp * P : (hp + 1) * P],
                                rhs=pt[:, co, :],
                                start=(co == 0),
                                stop=(co == CO - 1),
                            )
                        nc.scalar.copy(
                            qt_sb[:, hp, qc * 512 : (qc + 1) * 512], qp[:]
                        )

            def ph4_unit(qc, hp):
                # attention for one (query-half, head-pair): heads a=2hp,
                # b=2hp+1 run concurrently on PE row-groups 0/64 (scores)
                # and col-groups 0/64 (denominator ones-matmuls).
                kt_sb, v_sb, qt_sb = state["kt_sb"], state["v_sb"], state["qt_sb"]
                ot_sb = state["ot_sb"]
                qsl = slice(qc * 512, (qc + 1) * 512)
                ha, hb = 2 * hp, 2 * hp + 1
                otp = ps_ot.tile([P, 2, 512], F32, tag="ot")
                # two banks: interleaved accumulation groups must not
                # share a bank (start=True clears has_written bank-wide)
                dn = ps_dn.tile([P, 2, 512], F32, tag="dn")
                SKEW = 2
                pending = []

                def consume(ex, kvc):
                    # denominator pair: M=1, col-groups 0 / 64 -> concurrent
                    nc.tensor.matmul(
                        dn[0:1, 0, :],
                        lhsT=ones[:],
                        rhs=ex[:, 0, :],
                        start=(kvc == 0),
                        stop=(kvc == 15),
                    )
                    nc.tensor.matmul(
                        dn[64:65, 1, :],
                        lhsT=ones[:],
                        rhs=ex[:, 1, :],
                        start=(kvc == 0),
                        stop=(kvc == 15),
                    )
                    # P@V accumulation for both heads
                    nc.tensor.matmul(
                        otp[:, 0, :],
                        lhsT=v_sb[:, kvc, ha * DV : (ha + 1) * DV],
                        rhs=ex[:, 0, :],
                        start=(kvc == 0),
                        stop=(kvc == 15),
                    )
                    nc.tensor.matmul(
                        otp[:, 1, :],
                        lhsT=v_sb[:, kvc, hb * DV : (hb + 1) * DV],
                        rhs=ex[:, 1, :],
                        start=(kvc == 0),
                        stop=(kvc == 15),
                    )

                for kvc in range(16):
                    st = ps_st.tile([P, 2, 512], F32, tag="st")
                    for hh, base in ((0, 0), (1, 64)):
                        nc.tensor.matmul(
                            st[:, hh, :],
                            lhsT=kt_sb[
                                base : base + 64, hp, kvc * P : (kvc + 1) * P
                            ],
                            rhs=qt_sb[base : base + 64, hp, qsl],
                            start=True,
                            stop=True,
                        )
                    ex = exp_pool.tile([P, 2, 512], BF16, tag="exp")
                    nc.scalar.activation(
                        ex[:],
                        st[:],
                        mybir.ActivationFunctionType.Exp,
                        bias=0.0,
                        scale=float(EXP_SCALE),
                    )
                    pending.append((ex, kvc))
                    if len(pending) > SKEW:
                        consume(*pending.pop(0))
                for item in pending:
                    consume(*item)

                # normalize: denominators sit at psum partitions 0 (head a)
                # and 64 (head b). Broadcast via K=1 PE matmul (f32r), then
                # full-lane approx reciprocal + multiply. (gpsimd
                # partition_broadcast only works from partition 0 on HW.)
                dr = r_pool.tile([P, 512], F32R, tag="r")
                nc.vector.tensor_copy(dr[0:1, :], dn[0:1, 0, :])
                nc.vector.tensor_copy(dr[64:65, :], dn[64:65, 1, :])
                for hh, (h, base) in enumerate(((ha, 0), (hb, 64))):
                    dbp = ps_st.tile([P, 512], F32, tag="st")
                    nc.tensor.matmul(
                        dbp[:],
                        lhsT=ones_row[base : base + 1, :],
                        rhs=dr[base : base + 1, :],
                        start=True,
                        stop=True,
                    )
                    rb = rb_pool.tile([P, 512], F32, tag="rb")
                    nc.vector.reciprocal_approx_fast(out=rb[:], in_=dbp[:])
                    nc.vector.tensor_mul(ot_sb[:, h, qsl], otp[:, hh, :], rb[:])

            def ph5_half(qc):
                # Y[qc rows] = O^T.T @ Wo ; 1MB output DMAs
                ot_sb, wo_sb = state["ot_sb"], state["wo_sb"]
                for n2 in range(2):
                    nsl = slice(n2 * 512, (n2 + 1) * 512)
                    yb = y_pool.tile([P, 4, 512], F32, tag="y")
                    for qi in range(4):
                        qt = qc * 4 + qi
                        yp = ps_st.tile([P, 512], F32, tag="st")
                        for hc in range(CO):
                            nc.tensor.matmul(
                                yp[:],
                                lhsT=ot_sb[:, hc, qt * P : (qt + 1) * P],
                                rhs=wo_sb[:, hc, nsl],
                                start=(hc == 0),
                                stop=(hc == CO - 1),
                            )
                        nc.vector.tensor_copy(yb[:, qi, :], yp[:])
                    nc.sync.dma_start(
                        out.rearrange("(g p) n -> p g n", p=P)[
                            :, qc * 4 : (qc + 1) * 4, nsl
                        ],
                        yb[:],
                    )

            def ph4():
                ot_sb = ot_pool.tile([P, CO, SQH], BF16, tag="ot_sb")
                state["ot_sb"] = ot_sb
                for qc in range(2):
                    for hp in range(HP):
                        ph4_unit(qc, hp)
                    ph5_half(qc)

            phases = [("p1", ph1), ("p23", ph23), ("p4", ph4)]
            if loop_phase == "none":
                for _, f in phases:
                    f()
            elif loop_phase == "all":
                with tc.For_i(
                    0, n_val, 1, hint_engines=tuple(mybir.ALL_ENGINES)
                ) as _i:
                    for _, f in phases:
                        f()
            else:
                for name, f in phases:
                    if name == loop_phase:
                        with tc.For_i(
                            0, n_val, 1, hint_engines=tuple(mybir.ALL_ENGINES)
                        ) as _i:
                            f()
                    else:
                        f()
    nc.finalize()
    return nc


_NC_CACHE = None


def _get_nc():
    global _NC_CACHE
    if _NC_CACHE is None:
        _NC_CACHE = build()
    return _NC_CACHE


def run_sharded(inputs: dict, n_iters: int = 1):
    """Shard full inputs over 8 cores, run, gather full output.

    Returns (full_output [B,SQ,DM] f32, raw BassKernelResults).
    """
    enc_full = np.ascontiguousarray(np.asarray(inputs["encoder_output"], dtype=np.float32))
    pre_full = np.ascontiguousarray(np.asarray(inputs["pre_output"], dtype=np.float32))
    wq = np.ascontiguousarray(np.asarray(inputs["Wq"], dtype=np.float32))
    wk = np.ascontiguousarray(np.asarray(inputs["Wk"], dtype=np.float32))
    wv = np.ascontiguousarray(np.asarray(inputs["Wv"], dtype=np.float32))
    wo = np.ascontiguousarray(np.asarray(inputs["Wo"], dtype=np.float32))
    nit = np.array([[n_iters]], dtype=np.uint32)

    in_maps = []
    for c in range(N_CORES):
        b, qh = c // 2, c % 2
        in_maps.append(
            {
                "enc": enc_full[b],
                "pre": pre_full[b, qh * SQH : (qh + 1) * SQH],
                "wq": wq,
                "wk": wk,
                "wv": wv,
                "wo": wo,
                "n_it": nit,
            }
        )
    res = run_bass_kernel_spmd(_get_nc(), in_maps, list(range(N_CORES)))
    full = np.empty((B, SQ, DM), dtype=np.float32)
    for c in range(N_CORES):
        b, qh = c // 2, c % 2
        full[b, qh * SQH : (qh + 1) * SQH] = res.results[c]["out"]
    return full, res


def kernel(**inputs) -> np.ndarray:
    full, _ = run_sharded(inputs, n_iters=1)
    return full
